# revision 23
# baseline (speedup 1.0000x reference)
"""Trainium2 Bass kernel for nn_AbstractAffine (DeepPoly-style backsubstitution).

Math
----
Reference scans L=16 layers over relational bound state (xl, xu, bl, bu):
    pl = max(xl,0); nl = min(xl,0); pu = max(xu,0); nu = min(xu,0)
    bl += pl@c_lo + nl@c_up ;  bu += pu@c_up + nu@c_lo
    xl  = pl@A_lo + nl@A_up ;  xu  = pu@A_up + nu@A_lo
Using max(x,0)=(x+|x|)/2, min(x,0)=(x-|x|)/2 with S=(A_lo+A_up)/2,
Dm=(A_lo-A_up)/2:
    xl' = xl@S + |xl|@Dm ;  xu' = xu@S - |xu|@Dm
(same form for the bias vectors and for the final input-bound reduction).

At scan entry xl == xu == W^T exactly, so the first step degenerates to an
affine function of the inputs (xl1 = x0@S0 + |x0|@Dm0, xu1 = x0@S0 -
|x0|@Dm0, same for the bias row). That step is constant-folded into the
host-side input packing; the device runs layers 1..15. This removes layer
0's 4 MB of HBM traffic, which otherwise DMA-starves the PE at startup
(layer 0's halved matmul work cannot cover its DMA time, the resulting PE
gaps re-throttle the HAM clock and the next layers run at half rate).

Mapping
-------
Output-neuron rows are sharded over 8 cores (128 rows each). Each core keeps
its state TRANSPOSED (contraction index j on partitions), packed per j-chunk
as a (128, 512) tile [xlT | xuT | |xl|T | -|xu|T]. Every matmul uses an
A-matrix slice (natural HBM layout) as the stationary operand and the packed
state as the moving operand (free dim 256), so no on-chip transposes are
ever needed. A-matrices are fed in fp16 (host-cast; adds ~5e-4 rel err),
state in fp16, accumulation in fp32 PSUM.

The final input-bound reduction is interleaved into layer 15's close (lag-3
behind the per-chunk PSUM closes) and the initial bias row is embedded in
the DVE bias accumulator, so the tail after the last matmul is just two
tiny DVE ops + the output DMA.
"""

import numpy as np
from contextlib import ExitStack

import concourse.bass as bass
import concourse.tile as tile
from concourse import bacc, mybir
from concourse.bass_utils import run_bass_kernel_spmd

L = 16
D = 1024
NCORES = 8
R = D // NCORES          # rows per core
JC = D // 128            # j-chunks

A_DT = mybir.dt.float16
ST_DT = mybir.dt.float16
A_NP = np.float16
ST_NP = np.float16

_CACHE = {}


def _build():
    f32 = mybir.dt.float32
    nc = bacc.Bacc(None, target_bir_lowering=False)
    s_dram = nc.dram_tensor("s_mats", [L - 1, 128, JC * 1024], A_DT,
                            kind="ExternalInput")
    d_dram = nc.dram_tensor("d_mats", [L - 1, 128, JC * 1024], A_DT,
                            kind="ExternalInput")
    scdc_dram = nc.dram_tensor("scdc", [128, JC * (2 * L + 2)], A_DT,
                               kind="ExternalInput")
    init_dram = nc.dram_tensor("init_t", [128, JC * 512], ST_DT,
                               kind="ExternalInput")
    bias_dram = nc.dram_tensor("bias0", [1, 256], f32, kind="ExternalInput")
    out_dram = nc.dram_tensor("out", [1, 256], f32, kind="ExternalOutput")
    SDW = 2 * L + 2  # scdc columns per j-chunk

    MULT = mybir.AluOpType.mult
    MIN = mybir.AluOpType.min
    ADD = mybir.AluOpType.add

    with tile.TileContext(nc) as tc:
        with ExitStack() as ctx:
            apool = ctx.enter_context(tc.tile_pool(name="amat", bufs=12))
            spool = ctx.enter_context(tc.tile_pool(name="state", bufs=16))
            cpool = ctx.enter_context(tc.tile_pool(name="consts", bufs=1))
            ppool = ctx.enter_context(tc.tile_pool(name="psum", bufs=1, space="PSUM"))

            # PE warm-up: N=128 dummy matmuls into a trash PSUM bank while
            # the first DMAs are in flight — ~3.5-4.3us of sustained PE busy
            # releases the HAM clock throttle before the first real matmul
            dummy = cpool.tile([128, 128], A_DT, tag="dummy")
            nc.vector.memset(dummy[:], 0.0)
            trash = ppool.tile([128, 256], f32, tag="ps7", name="trash")
            NDUM = 52
            for i in range(NDUM):
                nc.tensor.matmul(trash[:, 0:128], dummy[:], dummy[:],
                                 start=(i == 0), stop=(i == NDUM - 1))

            # init state (layer-1 input, host-folded layer 0) on the ACT
            # HWDGE ring, interleaved with layer-1 d-chunks so both gate
            # streams make progress; the first quarter gates the first MMs
            t0_all = cpool.tile([128, JC * 512], ST_DT, tag="t0")

            # bias accumulator: ACC[p, :] += sc[p]*state[p, :] + dc[p]*abs[p, :]
            # summed over layers 1..15 on DVE (layer 0's contribution is
            # folded into the host-side bias row). The bias row is DMAd into
            # partition 0 so the single ones-vector fp32 matmul at the end
            # reduces everything at once.
            acc = cpool.tile([128, 256], f32, tag="acc")
            nc.vector.memset(acc[:], 0.0)
            ones = cpool.tile([128, 1], f32, tag="ones")
            nc.vector.memset(ones[:], 1.0)
            scdc_all = cpool.tile([128, JC * SDW], A_DT, tag="scdc")
            out_sb = cpool.tile([1, 256], f32, tag="outsb")
            scdc = [scdc_all[:, jc * SDW:(jc + 1) * SDW] for jc in range(JC)]

            T = [t0_all[:, jc * 512:(jc + 1) * 512] for jc in range(JC)]

            def bias_round(col_s, col_d):
                for jc in range(JC):
                    nc.vector.scalar_tensor_tensor(
                        acc[:], T[jc][:, 0:256],
                        scdc[jc][:, col_s:col_s + 1], acc[:], MULT, ADD)
                    nc.vector.scalar_tensor_tensor(
                        acc[:], T[jc][:, 256:512],
                        scdc[jc][:, col_d:col_d + 1], acc[:], MULT, ADD)

            def load_mat(l, mat, dram, eng, parts, aps, tag, bufs):
                # one tile + dma_start per (jc_start, jc_end) part; records a
                # per-jc [128,1024] view into aps
                for (p0, p1) in parts:
                    w = (p1 - p0) * 1024
                    t = apool.tile([128, w], A_DT, tag=tag,
                                   name=f"a{mat}{l}_{p0}", bufs=bufs)
                    eng.dma_start(t[:], dram[l - 1, :, p0 * 1024:p1 * 1024])
                    for jc in range(p0, p1):
                        aps[(mat, jc)] = t[:, (jc - p0) * 1024:(jc - p0 + 1) * 1024]

            pb1 = pb2 = None
            for l in range(1, L):
                # A-matrix loads. The DMA path is latency-bound at startup,
                # so the first layers use many fine-grained transfers (more
                # logical DMA queues in flight) split across both HWDGE
                # rings: layer-1 s-chunks on the SP ring, init-state +
                # layer-1 d-chunks interleaved on the ACT ring. Steady state
                # uses halves on SP (coarse enough to amortize the ~0.8us
                # per-dma_start sequencer issue cost).
                if l == 1:
                    # startup prefetch across THREE dma issuers: layer-1
                    # s-quarters on SP, d-quarters on ACT, init state +
                    # consts on the GPSIMD SWDGE ring; then layer-2 (s on
                    # SP, d on ACT) so its first chunks land before
                    # consumption
                    aps = {}
                    for h in range(4):
                        nc.gpsimd.dma_start(
                            t0_all[:, h * 1024:(h + 1) * 1024],
                            init_dram[:, h * 1024:(h + 1) * 1024])
                    nc.gpsimd.dma_start(scdc_all[:], scdc_dram[:])
                    nc.gpsimd.dma_start(acc[0:1, 0:256], bias_dram[:])
                    for h in range(JC):
                        load_mat(l, "s", s_dram, nc.sync, [(h, h + 1)], aps,
                                 "amat8", 16)
                        load_mat(l, "d", d_dram, nc.scalar, [(h, h + 1)], aps,
                                 "amat8", 16)
                    preloaded = {2: {}}
                    for (p0, p1) in ((0, 2), (2, 4), (4, 6), (6, 8)):
                        load_mat(2, "s", s_dram, nc.sync, [(p0, p1)],
                                 preloaded[2], "amat4", 16)
                        load_mat(2, "d", d_dram, nc.scalar, [(p0, p1)],
                                 preloaded[2], "amat4", 16)
                elif l == 2:
                    aps = preloaded[2]
                elif l == 3:
                    aps = {}
                    for (p0, p1) in ((0, 2), (2, 4), (4, 6), (6, 8)):
                        load_mat(l, "s", s_dram, nc.sync, [(p0, p1)], aps,
                                 "amat4", 16)
                        load_mat(l, "d", d_dram, nc.sync, [(p0, p1)], aps,
                                 "amat4", 16)
                else:
                    aps = {}
                    load_mat(l, "s", s_dram, nc.sync, [(0, 4), (4, 8)], aps,
                             "amath", 8)
                    load_mat(l, "d", d_dram, nc.sync, [(0, 4), (4, 8)], aps,
                             "amath", 8)

                def aslice(mat, jc):
                    return aps[(mat, jc)]

                bias_round(2 * l, 2 * l + 1)

                ps = [ppool.tile([128, 256], f32, tag=f"ps{i}", name=f"ps{i}_{l}")
                      for i in range(8)]
                newT = [spool.tile([128, 512], ST_DT, tag="T", name=f"T{j}_{l}")
                        for j in range(JC)]
                # jc-outer for all but the last j-chunk: consumes DMA chunks
                # as they land, all 8 cc accumulation groups open in their
                # own PSUM banks
                npad = 5 if l == 1 else (2 if l == 2 else 0)
                for jc in range(JC - 1):
                    for cc in range(8):
                        off = cc * 128
                        nc.tensor.matmul(
                            ps[cc][:], aslice("s", jc)[:, off:off + 128],
                            T[jc][:, 0:256], start=(jc == 0), stop=False)
                        nc.tensor.matmul(
                            ps[cc][:], aslice("d", jc)[:, off:off + 128],
                            T[jc][:, 256:512], start=False, stop=False)
                    # zero-contribution pad matmuls at the jc boundary: the
                    # startup layers are DMA-bound, and a PE idle gap while
                    # waiting for the next chunk re-throttles the HAM clock
                    # (everything after runs at half rate for ~3.4us). The
                    # pads (zeros stationary, 32 cols -> tiny LDWEIGHTS)
                    # keep the array streaming through the wait.
                    for _ in range(npad):
                        nc.tensor.matmul(
                            ps[0][0:32, 0:256], dummy[:, 0:32],
                            T[0][:, 0:256], start=False, stop=False)
                # last j-chunk cc-outer: staggers group closes so PSUM->SBUF
                # copies overlap the remaining matmuls
                jc = JC - 1
                last = (l == L - 1)
                for cc in range(8):
                    off = cc * 128
                    nc.tensor.matmul(
                        ps[cc][:], aslice("s", jc)[:, off:off + 128],
                        T[jc][:, 0:256], start=False, stop=False)
                    nc.tensor.matmul(
                        ps[cc][:], aslice("d", jc)[:, off:off + 128],
                        T[jc][:, 256:512], start=False, stop=True)
                    nt = newT[cc]
                    nc.scalar.copy(nt[:, 0:256], ps[cc][:])
                    nc.scalar.activation(
                        nt[:, 256:384], nt[:, 0:128],
                        mybir.ActivationFunctionType.Abs)
                    nc.vector.scalar_tensor_tensor(
                        nt[:, 384:512], nt[:, 128:256], -1.0, nt[:, 128:256],
                        MULT, MIN)
                    if last:
                        # final input-bound reduction, interleaved lag-3
                        # behind the chunk closes so the PE never waits on
                        # the PSUM->SBUF copy chain
                        if cc == 3:
                            pb2 = ppool.tile([1, 256], f32, tag="ps0",
                                             name="pb2")
                        if cc >= 3:
                            k = cc - 3
                            nc.tensor.matmul(
                                pb2[:], scdc[k][:, 2 * L:2 * L + 1],
                                newT[k][:, 0:256], start=(k == 0), stop=False)
                            nc.tensor.matmul(
                                pb2[:], scdc[k][:, 2 * L + 1:2 * L + 2],
                                newT[k][:, 256:512], start=False, stop=False)
                        if cc == 4:
                            # partition-reduce the bias accumulator (fp32
                            # matmul; acc was complete by mid-layer-15)
                            pb1 = ppool.tile([1, 256], f32, tag="ps1",
                                             name="pb1")
                            nc.tensor.matmul(pb1[:], ones[:], acc[:],
                                             start=True, stop=True)
                T = newT

            for k in range(5, JC):
                nc.tensor.matmul(pb2[:], scdc[k][:, 2 * L:2 * L + 1],
                                 T[k][:, 0:256], start=False, stop=False)
                nc.tensor.matmul(pb2[:], scdc[k][:, 2 * L + 1:2 * L + 2],
                                 T[k][:, 256:512], start=False, stop=(k == JC - 1))
            nc.vector.tensor_copy(out_sb[:], pb2[:])
            nc.vector.tensor_add(out_sb[:], out_sb[:], pb1[:])
            nc.sync.dma_start(out_dram[:], out_sb[:])
    nc.compile()
    return nc


def _prep_inputs(weights, biases, net_x_lowers, net_x_uppers,
                 net_b_lowers, net_b_uppers, input_lowers, input_uppers):
    W = np.ascontiguousarray(np.asarray(weights, dtype=np.float32))
    b = np.asarray(biases, dtype=np.float32).reshape(D)
    AL = np.asarray(net_x_lowers, dtype=np.float32)
    AU = np.asarray(net_x_uppers, dtype=np.float32)
    cL = np.asarray(net_b_lowers, dtype=np.float32).reshape(L, D)
    cU = np.asarray(net_b_uppers, dtype=np.float32).reshape(L, D)
    lo = np.asarray(input_lowers, dtype=np.float32).reshape(D)
    up = np.asarray(input_uppers, dtype=np.float32).reshape(D)

    S = 0.5 * (AL + AU)
    Dm = 0.5 * (AL - AU)
    # (L-1, 128, JC*1024): [l, p, jc*1024 + c] = S[l+1, jc*128 + p, c]
    s_mats = np.ascontiguousarray(
        S[1:].reshape(L - 1, JC, 128, D).transpose(0, 2, 1, 3)
        .reshape(L - 1, 128, JC * D)).astype(A_NP)
    d_mats = np.ascontiguousarray(
        Dm[1:].reshape(L - 1, JC, 128, D).transpose(0, 2, 1, 3)
        .reshape(L - 1, 128, JC * D)).astype(A_NP)

    sc = 0.5 * (cL + cU)
    dc = 0.5 * (cL - cU)
    s_in = 0.5 * (lo + up)
    d_in = 0.5 * (lo - up)
    sd = np.empty((JC, 128, 2 * L + 2), np.float32)
    sd[:, :, 0:2 * L:2] = sc.reshape(L, JC, 128).transpose(1, 2, 0)
    sd[:, :, 1:2 * L:2] = dc.reshape(L, JC, 128).transpose(1, 2, 0)
    sd[:, :, 2 * L] = s_in.reshape(JC, 128)
    sd[:, :, 2 * L + 1] = d_in.reshape(JC, 128)
    # dram layout (128, JC*SDW): [p, jc*SDW + col]
    scdc = np.ascontiguousarray(
        sd.transpose(1, 0, 2).reshape(128, JC * (2 * L + 2))).astype(A_NP)

    # host-folded layer 0: at scan entry xl == xu == W^T, so
    #   xl1 = x0@S0 + |x0|@Dm0,  xu1 = x0@S0 - |x0|@Dm0
    #   bl1 = b + x0@sc0 + |x0|@dc0,  bu1 = b + x0@sc0 - |x0|@dc0
    x0 = W.T
    p0 = x0 @ S[0]
    q0 = np.abs(x0) @ Dm[0]
    x1l = (p0 + q0).astype(ST_NP)   # (D rows, D cols), round once
    x1u = (p0 - q0).astype(ST_NP)
    x1la = np.abs(x1l)
    x1ua = -np.abs(x1u)
    bs = x0 @ sc[0]
    bd = np.abs(x0) @ dc[0]
    bl1 = b + bs + bd
    bu1 = b + bs - bd

    in_maps = []
    for k in range(NCORES):
        rs = slice(k * R, (k + 1) * R)
        initT = np.empty((JC, 128, 512), ST_NP)
        initT[:, :, 0:128] = x1l[rs].T.reshape(JC, 128, R)
        initT[:, :, 128:256] = x1u[rs].T.reshape(JC, 128, R)
        initT[:, :, 256:384] = x1la[rs].T.reshape(JC, 128, R)
        initT[:, :, 384:512] = x1ua[rs].T.reshape(JC, 128, R)
        # dram layout (128, JC*512): [p, jc*512 + c]
        initT = np.ascontiguousarray(
            initT.transpose(1, 0, 2).reshape(128, JC * 512))
        b0 = np.empty((1, 256), np.float32)
        b0[0, 0:128] = bl1[rs]
        b0[0, 128:256] = bu1[rs]
        in_maps.append({
            "s_mats": s_mats,
            "d_mats": d_mats,
            "scdc": scdc,
            "init_t": initT,
            "bias0": b0,
        })
    return in_maps


def _run(inputs, trace=False):
    if "nc" not in _CACHE:
        _CACHE["nc"] = _build()
    nc = _CACHE["nc"]
    in_maps = _prep_inputs(**inputs)
    try:
        res = run_bass_kernel_spmd(nc, in_maps, core_ids=list(range(NCORES)),
                                   trace=trace)
    except Exception:
        # transient NRT device errors have been observed; retry once
        res = run_bass_kernel_spmd(nc, in_maps, core_ids=list(range(NCORES)),
                                   trace=trace)
    lowers = np.empty((D, 1), np.float32)
    uppers = np.empty((D, 1), np.float32)
    for k in range(NCORES):
        arr = res.results[k]["out"]
        lowers[k * R:(k + 1) * R, 0] = arr[0, 0:128]
        uppers[k * R:(k + 1) * R, 0] = arr[0, 128:256]
    out = np.stack([lowers, uppers])
    return out, res


def kernel(**inputs):
    out, _ = _run(inputs, trace=False)
    return out


# revision 24
# speedup vs baseline: 1.0249x; 1.0249x over previous
"""Trainium2 Bass kernel for nn_AbstractAffine (DeepPoly-style backsubstitution).

Math
----
Reference scans L=16 layers over relational bound state (xl, xu, bl, bu):
    pl = max(xl,0); nl = min(xl,0); pu = max(xu,0); nu = min(xu,0)
    bl += pl@c_lo + nl@c_up ;  bu += pu@c_up + nu@c_lo
    xl  = pl@A_lo + nl@A_up ;  xu  = pu@A_up + nu@A_lo
Using max(x,0)=(x+|x|)/2, min(x,0)=(x-|x|)/2 with S=(A_lo+A_up)/2,
Dm=(A_lo-A_up)/2:
    xl' = xl@S + |xl|@Dm ;  xu' = xu@S - |xu|@Dm
(same form for the bias vectors and for the final input-bound reduction).

At scan entry xl == xu == W^T exactly, so the first step degenerates to an
affine function of the inputs (xl1 = x0@S0 + |x0|@Dm0, xu1 = x0@S0 -
|x0|@Dm0, same for the bias row). That step is constant-folded into the
host-side input packing; the device runs layers 1..15. This removes layer
0's 4 MB of HBM traffic, which otherwise DMA-starves the PE at startup
(layer 0's halved matmul work cannot cover its DMA time, the resulting PE
gaps re-throttle the HAM clock and the next layers run at half rate).

Mapping
-------
Output-neuron rows are sharded over 8 cores (128 rows each). Each core keeps
its state TRANSPOSED (contraction index j on partitions), packed per j-chunk
as a (128, 512) tile [xlT | xuT | |xl|T | -|xu|T]. Every matmul uses an
A-matrix slice (natural HBM layout) as the stationary operand and the packed
state as the moving operand (free dim 256), so no on-chip transposes are
ever needed. A-matrices are fed in fp16 (host-cast; adds ~5e-4 rel err),
state in fp16, accumulation in fp32 PSUM.

The final input-bound reduction is interleaved into layer 15's close (lag-3
behind the per-chunk PSUM closes) and the initial bias row is embedded in
the DVE bias accumulator, so the tail after the last matmul is just two
tiny DVE ops + the output DMA.
"""

import numpy as np
from contextlib import ExitStack

import concourse.bass as bass
import concourse.tile as tile
from concourse import bacc, mybir
from concourse.bass_utils import run_bass_kernel_spmd

L = 16
D = 1024
NCORES = 8
R = D // NCORES          # rows per core
JC = D // 128            # j-chunks

A_DT = mybir.dt.float16
ST_DT = mybir.dt.float16
A_NP = np.float16
ST_NP = np.float16

_CACHE = {}


def _build():
    f32 = mybir.dt.float32
    nc = bacc.Bacc(None, target_bir_lowering=False)
    s_dram = nc.dram_tensor("s_mats", [L - 1, 128, JC * 1024], A_DT,
                            kind="ExternalInput")
    d_dram = nc.dram_tensor("d_mats", [L - 1, 128, JC * 1024], A_DT,
                            kind="ExternalInput")
    scdc_dram = nc.dram_tensor("scdc", [128, JC * (2 * L + 2)], A_DT,
                               kind="ExternalInput")
    init_dram = nc.dram_tensor("init_t", [128, JC * 512], ST_DT,
                               kind="ExternalInput")
    bias_dram = nc.dram_tensor("bias0", [1, 256], f32, kind="ExternalInput")
    out_dram = nc.dram_tensor("out", [1, 256], f32, kind="ExternalOutput")
    SDW = 2 * L + 2  # scdc columns per j-chunk

    MULT = mybir.AluOpType.mult
    MIN = mybir.AluOpType.min
    ADD = mybir.AluOpType.add

    with tile.TileContext(nc) as tc:
        with ExitStack() as ctx:
            apool = ctx.enter_context(tc.tile_pool(name="amat", bufs=12))
            spool = ctx.enter_context(tc.tile_pool(name="state", bufs=16))
            cpool = ctx.enter_context(tc.tile_pool(name="consts", bufs=1))
            ppool = ctx.enter_context(tc.tile_pool(name="psum", bufs=1, space="PSUM"))

            # PE warm-up: N=128 dummy matmuls into a trash PSUM bank while
            # the first DMAs are in flight — ~3.5-4.3us of sustained PE busy
            # releases the HAM clock throttle before the first real matmul
            dummy = cpool.tile([128, 128], A_DT, tag="dummy")
            nc.vector.memset(dummy[:], 0.0)
            trash = ppool.tile([128, 256], f32, tag="ps7", name="trash")
            NDUM = 52
            for i in range(NDUM):
                nc.tensor.matmul(trash[:, 0:128], dummy[:], dummy[:],
                                 start=(i == 0), stop=(i == NDUM - 1))

            # init state (layer-1 input, host-folded layer 0) on the ACT
            # HWDGE ring, interleaved with layer-1 d-chunks so both gate
            # streams make progress; the first quarter gates the first MMs
            t0_all = cpool.tile([128, JC * 512], ST_DT, tag="t0")

            # bias accumulator: ACC[p, :] += sc[p]*state[p, :] + dc[p]*abs[p, :]
            # summed over layers 1..15 on DVE (layer 0's contribution is
            # folded into the host-side bias row). The bias row is DMAd into
            # partition 0 so the single ones-vector fp32 matmul at the end
            # reduces everything at once.
            acc = cpool.tile([128, 256], f32, tag="acc")
            nc.vector.memset(acc[:], 0.0)
            ones = cpool.tile([128, 1], f32, tag="ones")
            nc.vector.memset(ones[:], 1.0)
            scdc_all = cpool.tile([128, JC * SDW], A_DT, tag="scdc")
            out_sb = cpool.tile([1, 256], f32, tag="outsb")
            scdc = [scdc_all[:, jc * SDW:(jc + 1) * SDW] for jc in range(JC)]

            T = [t0_all[:, jc * 512:(jc + 1) * 512] for jc in range(JC)]

            def bias_round(col_s, col_d):
                for jc in range(JC):
                    nc.vector.scalar_tensor_tensor(
                        acc[:], T[jc][:, 0:256],
                        scdc[jc][:, col_s:col_s + 1], acc[:], MULT, ADD)
                    nc.vector.scalar_tensor_tensor(
                        acc[:], T[jc][:, 256:512],
                        scdc[jc][:, col_d:col_d + 1], acc[:], MULT, ADD)

            def load_mat(l, mat, dram, eng, parts, aps, tag, bufs):
                # one tile + dma_start per (jc_start, jc_end) part; records a
                # per-jc [128,1024] view into aps
                for (p0, p1) in parts:
                    w = (p1 - p0) * 1024
                    t = apool.tile([128, w], A_DT, tag=tag,
                                   name=f"a{mat}{l}_{p0}", bufs=bufs)
                    eng.dma_start(t[:], dram[l - 1, :, p0 * 1024:p1 * 1024])
                    for jc in range(p0, p1):
                        aps[(mat, jc)] = t[:, (jc - p0) * 1024:(jc - p0 + 1) * 1024]

            pb1 = pb2 = None
            for l in range(1, L):
                # A-matrix loads. The DMA path is latency-bound at startup,
                # so the first layers use many fine-grained transfers (more
                # logical DMA queues in flight) split across both HWDGE
                # rings: layer-1 s-chunks on the SP ring, init-state +
                # layer-1 d-chunks interleaved on the ACT ring. Steady state
                # uses halves on SP (coarse enough to amortize the ~0.8us
                # per-dma_start sequencer issue cost).
                if l == 1:
                    # startup prefetch across THREE dma issuers: layer-1
                    # s-quarters on SP, d-quarters on ACT, init state +
                    # consts on the GPSIMD SWDGE ring; then layer-2 (s on
                    # SP, d on ACT) so its first chunks land before
                    # consumption
                    aps = {}
                    for h in range(4):
                        nc.gpsimd.dma_start(
                            t0_all[:, h * 1024:(h + 1) * 1024],
                            init_dram[:, h * 1024:(h + 1) * 1024])
                    nc.gpsimd.dma_start(scdc_all[:], scdc_dram[:])
                    nc.gpsimd.dma_start(acc[0:1, 0:256], bias_dram[:])
                    for h in range(JC):
                        load_mat(l, "s", s_dram, nc.sync, [(h, h + 1)], aps,
                                 "amat8", 16)
                        load_mat(l, "d", d_dram, nc.scalar, [(h, h + 1)], aps,
                                 "amat8", 16)
                    preloaded = {2: {}}
                    for (p0, p1) in ((0, 2), (2, 4), (4, 6), (6, 8)):
                        load_mat(2, "s", s_dram, nc.sync, [(p0, p1)],
                                 preloaded[2], "amat4", 16)
                        load_mat(2, "d", d_dram, nc.scalar, [(p0, p1)],
                                 preloaded[2], "amat4", 16)
                elif l == 2:
                    aps = preloaded[2]
                elif l == 3:
                    aps = {}
                    for (p0, p1) in ((0, 2), (2, 4), (4, 6), (6, 8)):
                        load_mat(l, "s", s_dram, nc.sync, [(p0, p1)], aps,
                                 "amat4", 16)
                        load_mat(l, "d", d_dram, nc.sync, [(p0, p1)], aps,
                                 "amat4", 16)
                else:
                    aps = {}
                    load_mat(l, "s", s_dram, nc.sync, [(0, 4), (4, 8)], aps,
                             "amath", 8)
                    load_mat(l, "d", d_dram, nc.sync, [(0, 4), (4, 8)], aps,
                             "amath", 8)

                def aslice(mat, jc):
                    return aps[(mat, jc)]

                bias_round(2 * l, 2 * l + 1)

                ps = [ppool.tile([128, 256], f32, tag=f"ps{i}", name=f"ps{i}_{l}")
                      for i in range(8)]
                newT = [spool.tile([128, 512], ST_DT, tag="T", name=f"T{j}_{l}")
                        for j in range(JC)]
                # jc-outer for all but the last j-chunk: consumes DMA chunks
                # as they land, all 8 cc accumulation groups open in their
                # own PSUM banks
                npad = 0
                for jc in range(JC - 1):
                    for cc in range(8):
                        off = cc * 128
                        nc.tensor.matmul(
                            ps[cc][:], aslice("s", jc)[:, off:off + 128],
                            T[jc][:, 0:256], start=(jc == 0), stop=False)
                        nc.tensor.matmul(
                            ps[cc][:], aslice("d", jc)[:, off:off + 128],
                            T[jc][:, 256:512], start=False, stop=False)
                    # zero-contribution pad matmuls at the jc boundary: the
                    # startup layers are DMA-bound, and a PE idle gap while
                    # waiting for the next chunk re-throttles the HAM clock
                    # (everything after runs at half rate for ~3.4us). The
                    # pads (zeros stationary, 32 cols -> tiny LDWEIGHTS)
                    # keep the array streaming through the wait.
                    for _ in range(npad):
                        nc.tensor.matmul(
                            ps[0][0:32, 0:256], dummy[:, 0:32],
                            T[0][:, 0:256], start=False, stop=False)
                # last j-chunk cc-outer: staggers group closes so PSUM->SBUF
                # copies overlap the remaining matmuls
                jc = JC - 1
                last = (l == L - 1)
                for cc in range(8):
                    off = cc * 128
                    nc.tensor.matmul(
                        ps[cc][:], aslice("s", jc)[:, off:off + 128],
                        T[jc][:, 0:256], start=False, stop=False)
                    nc.tensor.matmul(
                        ps[cc][:], aslice("d", jc)[:, off:off + 128],
                        T[jc][:, 256:512], start=False, stop=True)
                    nt = newT[cc]
                    nc.scalar.copy(nt[:, 0:256], ps[cc][:])
                    nc.scalar.activation(
                        nt[:, 256:384], nt[:, 0:128],
                        mybir.ActivationFunctionType.Abs)
                    nc.vector.scalar_tensor_tensor(
                        nt[:, 384:512], nt[:, 128:256], -1.0, nt[:, 128:256],
                        MULT, MIN)
                    if last:
                        # final input-bound reduction, interleaved lag-3
                        # behind the chunk closes so the PE never waits on
                        # the PSUM->SBUF copy chain
                        if cc == 3:
                            pb2 = ppool.tile([1, 256], f32, tag="ps0",
                                             name="pb2")
                        if cc >= 3:
                            k = cc - 3
                            nc.tensor.matmul(
                                pb2[:], scdc[k][:, 2 * L:2 * L + 1],
                                newT[k][:, 0:256], start=(k == 0), stop=False)
                            nc.tensor.matmul(
                                pb2[:], scdc[k][:, 2 * L + 1:2 * L + 2],
                                newT[k][:, 256:512], start=False, stop=False)
                        if cc == 4:
                            # partition-reduce the bias accumulator (fp32
                            # matmul; acc was complete by mid-layer-15)
                            pb1 = ppool.tile([1, 256], f32, tag="ps1",
                                             name="pb1")
                            nc.tensor.matmul(pb1[:], ones[:], acc[:],
                                             start=True, stop=True)
                T = newT

            for k in range(5, JC):
                nc.tensor.matmul(pb2[:], scdc[k][:, 2 * L:2 * L + 1],
                                 T[k][:, 0:256], start=False, stop=False)
                nc.tensor.matmul(pb2[:], scdc[k][:, 2 * L + 1:2 * L + 2],
                                 T[k][:, 256:512], start=False, stop=(k == JC - 1))
            nc.vector.tensor_copy(out_sb[:], pb2[:])
            nc.vector.tensor_add(out_sb[:], out_sb[:], pb1[:])
            nc.sync.dma_start(out_dram[:], out_sb[:])
    nc.compile()
    return nc


def _prep_inputs(weights, biases, net_x_lowers, net_x_uppers,
                 net_b_lowers, net_b_uppers, input_lowers, input_uppers):
    W = np.ascontiguousarray(np.asarray(weights, dtype=np.float32))
    b = np.asarray(biases, dtype=np.float32).reshape(D)
    AL = np.asarray(net_x_lowers, dtype=np.float32)
    AU = np.asarray(net_x_uppers, dtype=np.float32)
    cL = np.asarray(net_b_lowers, dtype=np.float32).reshape(L, D)
    cU = np.asarray(net_b_uppers, dtype=np.float32).reshape(L, D)
    lo = np.asarray(input_lowers, dtype=np.float32).reshape(D)
    up = np.asarray(input_uppers, dtype=np.float32).reshape(D)

    S = 0.5 * (AL + AU)
    Dm = 0.5 * (AL - AU)
    # (L-1, 128, JC*1024): [l, p, jc*1024 + c] = S[l+1, jc*128 + p, c]
    s_mats = np.ascontiguousarray(
        S[1:].reshape(L - 1, JC, 128, D).transpose(0, 2, 1, 3)
        .reshape(L - 1, 128, JC * D)).astype(A_NP)
    d_mats = np.ascontiguousarray(
        Dm[1:].reshape(L - 1, JC, 128, D).transpose(0, 2, 1, 3)
        .reshape(L - 1, 128, JC * D)).astype(A_NP)

    sc = 0.5 * (cL + cU)
    dc = 0.5 * (cL - cU)
    s_in = 0.5 * (lo + up)
    d_in = 0.5 * (lo - up)
    sd = np.empty((JC, 128, 2 * L + 2), np.float32)
    sd[:, :, 0:2 * L:2] = sc.reshape(L, JC, 128).transpose(1, 2, 0)
    sd[:, :, 1:2 * L:2] = dc.reshape(L, JC, 128).transpose(1, 2, 0)
    sd[:, :, 2 * L] = s_in.reshape(JC, 128)
    sd[:, :, 2 * L + 1] = d_in.reshape(JC, 128)
    # dram layout (128, JC*SDW): [p, jc*SDW + col]
    scdc = np.ascontiguousarray(
        sd.transpose(1, 0, 2).reshape(128, JC * (2 * L + 2))).astype(A_NP)

    # host-folded layer 0: at scan entry xl == xu == W^T, so
    #   xl1 = x0@S0 + |x0|@Dm0,  xu1 = x0@S0 - |x0|@Dm0
    #   bl1 = b + x0@sc0 + |x0|@dc0,  bu1 = b + x0@sc0 - |x0|@dc0
    x0 = W.T
    p0 = x0 @ S[0]
    q0 = np.abs(x0) @ Dm[0]
    x1l = (p0 + q0).astype(ST_NP)   # (D rows, D cols), round once
    x1u = (p0 - q0).astype(ST_NP)
    x1la = np.abs(x1l)
    x1ua = -np.abs(x1u)
    bs = x0 @ sc[0]
    bd = np.abs(x0) @ dc[0]
    bl1 = b + bs + bd
    bu1 = b + bs - bd

    in_maps = []
    for k in range(NCORES):
        rs = slice(k * R, (k + 1) * R)
        initT = np.empty((JC, 128, 512), ST_NP)
        initT[:, :, 0:128] = x1l[rs].T.reshape(JC, 128, R)
        initT[:, :, 128:256] = x1u[rs].T.reshape(JC, 128, R)
        initT[:, :, 256:384] = x1la[rs].T.reshape(JC, 128, R)
        initT[:, :, 384:512] = x1ua[rs].T.reshape(JC, 128, R)
        # dram layout (128, JC*512): [p, jc*512 + c]
        initT = np.ascontiguousarray(
            initT.transpose(1, 0, 2).reshape(128, JC * 512))
        b0 = np.empty((1, 256), np.float32)
        b0[0, 0:128] = bl1[rs]
        b0[0, 128:256] = bu1[rs]
        in_maps.append({
            "s_mats": s_mats,
            "d_mats": d_mats,
            "scdc": scdc,
            "init_t": initT,
            "bias0": b0,
        })
    return in_maps


def _run(inputs, trace=False):
    if "nc" not in _CACHE:
        _CACHE["nc"] = _build()
    nc = _CACHE["nc"]
    in_maps = _prep_inputs(**inputs)
    try:
        res = run_bass_kernel_spmd(nc, in_maps, core_ids=list(range(NCORES)),
                                   trace=trace)
    except Exception:
        # transient NRT device errors have been observed; retry once
        res = run_bass_kernel_spmd(nc, in_maps, core_ids=list(range(NCORES)),
                                   trace=trace)
    lowers = np.empty((D, 1), np.float32)
    uppers = np.empty((D, 1), np.float32)
    for k in range(NCORES):
        arr = res.results[k]["out"]
        lowers[k * R:(k + 1) * R, 0] = arr[0, 0:128]
        uppers[k * R:(k + 1) * R, 0] = arr[0, 128:256]
    out = np.stack([lowers, uppers])
    return out, res


def kernel(**inputs):
    out, _ = _run(inputs, trace=False)
    return out
